# revision 1
# baseline (speedup 1.0000x reference)
"""BinaryLayerWrapper (sync-BN + sign + binarized 3x3 conv) on 8 TRN2 cores.

Strategy (data-parallel, per sharding hint):
  - shard batch B=32 -> 4 images per core; conv weights replicated
  - phase A: stream x shard to SBUF (kept resident), per-channel partial
    sums sum(x), sum(x^2) over local batch+space; weight prep overlapped
    (alpha = mean|w|, sign(w), PE-transpose to [Cin,Cout] tiles)
  - tiny AllReduce (add) of [128,4] partial stats across 8 cores (sync-BN)
  - per-channel a = gamma*rsqrt(var+eps), b = beta - mean*a
  - phase C: xb = Sign(a*x+b) in fp8/bf16 written into zero-padded 58x58
    planes; 3x3 conv = 9 (fp8 DoubleRow) or 18 (bf16) accumulated matmuls
    per output tile on the PE array (N=464 = 8 output rows x 58 padded
    cols), then scale by alpha and DMA the valid interior out.

The conv math is exact: xb is +-1 (exact in fp8e4m3/bf16), sign(w) is
+-1, products accumulate in fp32 PSUM as small integers; alpha scaling
happens once at the end.

Instruction emission order is engine-FIFO-aware: engines execute their
queues in program order, so the sync-BN critical chain (stats ->
allreduce -> coefs -> first sign) is emitted before bulk weight-prep
work on the same engines.
"""

import os
from contextlib import ExitStack

import numpy as np

from concourse import bacc, bass, masks, mybir, tile
from concourse.bass_utils import run_bass_kernel_spmd

F32 = mybir.dt.float32
BF16 = mybir.dt.bfloat16
FP8 = mybir.dt.float8e4

# fp8e4m3 + DoubleRow (2 Cin chunks per matmul pass)
USE_FP8 = os.environ.get("USE_FP8", "1") == "1"

N_CORES = 8
B_LOC = 4          # images per core (32 / 8)
C = 256            # channels (in == out)
KC = 2             # 128-partition channel chunks
H = W = 56
PIX = H * W        # 3136
WP = W + 2         # 58 padded width
PLANE = WP * (H + 2)          # 58*58 = 3364
XBP_LEN = PLANE + 2           # +1 lead pad so all tap offsets are >= 0
PLANE_PAD = 3376              # XBP_LEN rounded to 16 (fp8 DoubleRow Ko step)
R = 8                         # output rows per matmul tile (N=464, 1 PSUM bank)
NF = R * WP                   # 464 matmul free dim
N_TOTAL = 32 * PIX            # full-batch elements per channel (sync-BN)


def build_program(num_devices: int = N_CORES, cc: bool = True,
                  stage: int = 3) -> bass.Bass:
    nc = bacc.Bacc("TRN2", target_bir_lowering=False, debug=False,
                   num_devices=num_devices)
    nc._use_cc = cc
    nc._cc_devices = num_devices
    nc._stage = stage

    x = nc.dram_tensor("x", [B_LOC, C, H, W], F32, kind="ExternalInput").ap()
    w = nc.dram_tensor("weight", [C, C, 3, 3], F32, kind="ExternalInput").ap()
    gamma = nc.dram_tensor("gamma", [C], F32, kind="ExternalInput").ap()
    beta = nc.dram_tensor("beta", [C], F32, kind="ExternalInput").ap()
    y = nc.dram_tensor("y", [B_LOC, C, H, W], F32, kind="ExternalOutput").ap()

    with tile.TileContext(nc) as tc:
        _body(tc, y, x, w, gamma, beta)
    nc.compile()
    return nc


def _body(tc: tile.TileContext, y, x, w, gamma, beta):
    nc = tc.nc
    add = mybir.AluOpType.add
    AF = mybir.ActivationFunctionType

    with (
        tc.tile_pool(name="singles", bufs=1) as singles,
        tc.tile_pool(name="wsbuf", bufs=1) as wspool,
        tc.tile_pool(name="xres", bufs=1) as xpool,
        tc.tile_pool(name="dram", bufs=1, space="DRAM") as dram,
    ):
        identity = singles.tile([128, 128], BF16, tag="identity")
        masks.make_identity(nc, identity[:])

        gb = singles.tile([128, 4], F32, tag="gb")  # cols: gamma k0,k1, beta k0,k1

        # per-(b,k,half) stat partials; cols indexed (k*B_LOC + b)*2 + h
        psum_parts = singles.tile([128, KC * B_LOC * 2], F32, tag="psum_parts")
        psq_parts = singles.tile([128, KC * B_LOC * 2], F32, tag="psq_parts")
        stats_local = singles.tile([128, 4], F32, tag="stats_local")
        gstats = singles.tile([128, 4], F32, tag="gstats")
        alpha = singles.tile([128, 2], F32, tag="alpha")        # per-o-chunk alpha
        alpha_raw = singles.tile([128, 2], F32, tag="alpha_raw")
        coefs = singles.tile([128, 12], F32, tag="coefs")       # scratch cols
        ab = singles.tile([128, 4], F32, tag="ab")  # cols: a k0,k1, b k0,k1

        # resident x shard: one [128, PIX] f32 tile per (b, k)
        xs = [[xpool.tile([128, PIX], F32, tag=f"xs{b}_{k}", name=f"xs{b}_{k}")
               for k in range(KC)] for b in range(B_LOC)]
        ws = {}

        # phase-C pools opened upfront: no SBUF/stack aliasing with the
        # phase-A scratch pools means no WAR deps delaying the border
        # memsets or the first conv matmuls
        phase_c_pools = ExitStack()
        xbpool = phase_c_pools.enter_context(tc.tile_pool(name="xbp", bufs=1))
        stpool = phase_c_pools.enter_context(tc.tile_pool(name="stage", bufs=8))
        cpsum = phase_c_pools.enter_context(
            tc.tile_pool(name="cpsum", bufs=6, space="PSUM"))

        if USE_FP8:
            xbp = [xbpool.tile([128, KC * PLANE_PAD], FP8, tag=f"xbp{b}",
                               name=f"xbp{b}")
                   for b in range(B_LOC)]
        else:
            xbp = [[xbpool.tile([128, XBP_LEN], BF16, tag=f"xbp{b}_{k}",
                                name=f"xbp{b}_{k}")
                    for k in range(KC)] for b in range(B_LOC)]

        # zero only the halo borders (the interior is fully overwritten
        # by the sign pass; the inter-plane alignment gap is never read)
        def memset_borders(t, base):
            nc.gpsimd.memset(t[:, base:base + 1], 0.0)           # lead elem
            nc.gpsimd.memset(t[:, base + 1:base + 1 + WP], 0.0)  # top row
            nc.gpsimd.memset(t[:, base + 1 + 57 * WP:base + 1 + 57 * WP + WP],
                             0.0)                                # bottom row
            side = (t[:, base + 1 + WP:base + 1 + 57 * WP]
                    .rearrange("p (h w) -> p h w", w=WP))
            nc.gpsimd.memset(side[:, :, 0:1], 0.0)               # left col
            nc.gpsimd.memset(side[:, :, WP - 1:WP], 0.0)         # right col
            nc.gpsimd.memset(t[:, base + 1 + PLANE:base + 1 + PLANE + 1], 0.0)

        for b in range(B_LOC):
            if USE_FP8:
                for k in range(KC):
                    memset_borders(xbp[b], k * PLANE_PAD)
            else:
                for k in range(KC):
                    memset_borders(xbp[b][k], 0)

        with (
            tc.tile_pool(name="wraw", bufs=1) as wraw_pool,
            tc.tile_pool(name="scr", bufs=3) as scr,
            tc.tile_pool(name="tpps", bufs=2, space="PSUM") as tp_psum,
        ):
            # ---- phase A: x shard DMA (first in the HWDGE queue — it
            # gates the sync-BN chain) + per-half-tile stats so the stat ops
            # trail the DMA stream by half a tile ----
            HPIX = PIX // 2
            for b in range(B_LOC):
                for k in range(KC):
                    for hf in range(2):
                        nc.sync.dma_start(
                            out=xs[b][k][:, hf * HPIX:(hf + 1) * HPIX],
                            in_=x[b, k * 128:(k + 1) * 128]
                            .rearrange("c h w -> c (h w)")[:, hf * HPIX:(hf + 1) * HPIX])
                        col = (k * B_LOC + b) * 2 + hf
                        xsl = xs[b][k][:, hf * HPIX:(hf + 1) * HPIX]
                        sa = scr.tile([128, HPIX], BF16, tag="scr_a", name="scr_a")
                        nc.scalar.activation(sa[:], xsl, AF.Copy,
                                             accum_out=psum_parts[:, col:col + 1])
                        sb = scr.tile([128, HPIX], BF16, tag="scr_b", name="scr_b")
                        nc.vector.scalar_tensor_tensor(
                            out=sb[:], in0=xsl, scalar=1.0, in1=xsl,
                            op0=mybir.AluOpType.mult, op1=mybir.AluOpType.mult,
                            accum_out=psq_parts[:, col:col + 1])
                        # HAM keep-warm: a discarded transpose gated on this
                        # half-tile's stat scratch paces PE activity through
                        # the DMA phase so the conv starts at the full clock
                        warm = tp_psum.tile([128, 128], BF16, tag="tp",
                                            name="warm")
                        nc.tensor.transpose(warm[:], sa[:, 0:128], identity[:])

            # one more keep-warm gated on the final stat scratch (~43us) to
            # narrow the PE-idle bridge before the weight transposes
            warm2 = tp_psum.tile([128, 128], BF16, tag="tp", name="warm2")
            nc.tensor.transpose(warm2[:], sb[:, 0:128], identity[:])

            # gamma/beta after the x stream so they don't delay it
            nc.sync.dma_start(out=gb[:, 0:2],
                              in_=gamma.rearrange("(k p) -> p k", p=128))
            nc.sync.dma_start(out=gb[:, 2:4],
                              in_=beta.rearrange("(k p) -> p k", p=128))

            # ---- finalize local stats + sync-BN all-reduce ----
            nc.vector.tensor_reduce(
                out=stats_local[:, 0:2],
                in_=psum_parts[:].rearrange("p (k bh) -> p k bh", k=KC),
                axis=mybir.AxisListType.X, op=add)
            nc.vector.tensor_reduce(
                out=stats_local[:, 2:4],
                in_=psq_parts[:].rearrange("p (k bh) -> p k bh", k=KC),
                axis=mybir.AxisListType.X, op=add)

            ccin = dram.tile([128, 4], F32, tag="ccin", name="ccin")
            ccout = dram.tile([128, 4], F32, tag="ccout", name="ccout")
            nc.sync.dma_start(out=ccin[:], in_=stats_local[:])
            if nc._use_cc:
                nc.gpsimd.collective_compute(
                    "AllReduce", add,
                    replica_groups=[list(range(nc._cc_devices))],
                    ins=[ccin.opt()], outs=[ccout.opt()])
            else:
                nc.sync.dma_start(out=ccout[:], in_=ccin[:])
            nc.sync.dma_start(out=gstats[:], in_=ccout[:])

            # ---- weight DMA + cheap weight math (fills engine idle while
            # the allreduce round-trips; w DMAs queue after x on HWDGE) ----
            wraws, wsigns = [], []
            for oc in range(2):
                wraw = wraw_pool.tile([128, C * 9], F32, tag=f"wraw{oc}",
                                      name=f"wraw{oc}")
                nc.sync.dma_start(
                    out=wraw[:],
                    in_=w[oc * 128:(oc + 1) * 128].rearrange("o c kh kw -> o (c kh kw)"))
                wsign = wraw_pool.tile([128, C * 9], BF16, tag=f"wsign{oc}",
                                       name=f"wsign{oc}")
                nc.scalar.activation(wsign[:], wraw[:], AF.Sign)
                wraws.append(wraw)
                wsigns.append(wsign)

            # ---- BN coefficients: a = gamma*inv, b = beta - mean*a ----
            mm = coefs[:, 0:4]       # mean k0,k1 | msq k0,k1
            mean = coefs[:, 0:2]
            msq = coefs[:, 2:4]
            m2 = coefs[:, 4:6]
            var = coefs[:, 6:8]
            rec = coefs[:, 8:10]
            inv = coefs[:, 10:12]
            # tiny [128,2] chain ops go on the otherwise-idle gpsimd queue so
            # they aren't latency-interleaved with bulk DVE work; only
            # reciprocal (DVE-only) and Sqrt (ACT) leave it
            nc.vector.tensor_scalar_mul(mm, gstats[:], 1.0 / N_TOTAL)
            nc.gpsimd.tensor_tensor(out=m2, in0=mean, in1=mean,
                                    op=mybir.AluOpType.mult)
            # var+eps = (msq + eps) - mean^2 in one op
            nc.vector.scalar_tensor_tensor(
                out=var, in0=msq, scalar=1e-5, in1=m2,
                op0=add, op1=mybir.AluOpType.subtract)
            nc.vector.reciprocal(rec, var)
            nc.scalar.activation(inv, rec, AF.Sqrt)
            nc.gpsimd.tensor_tensor(out=ab[:, 0:2], in0=gb[:, 0:2], in1=inv,
                                    op=mybir.AluOpType.mult)
            nc.gpsimd.tensor_tensor(out=ab[:, 2:4], in0=mean, in1=ab[:, 0:2],
                                    op=mybir.AluOpType.mult)
            nc.gpsimd.tensor_tensor(out=ab[:, 2:4], in0=gb[:, 2:4], in1=ab[:, 2:4],
                                    op=mybir.AluOpType.subtract)

            # alpha = mean|w| per output chunk (after the coef chain so these
            # big reduces don't block it in the DVE queue)
            for oc in range(2):
                nc.vector.tensor_reduce(
                    out=alpha_raw[:, oc:oc + 1], in_=wraws[oc][:],
                    axis=mybir.AxisListType.X, op=add, apply_absolute_value=True)
            nc.vector.tensor_scalar_mul(alpha[:], alpha_raw[:], 1.0 / (C * 9))

            # ---- weight transposes to [Cin, Cout] lhsT tiles (PE + DVE
            # copies; emitted after the coef chain so the small coef ops
            # aren't stuck behind 36 copies in the DVE queue) ----
            for oc in range(2):
                wsign3 = wsigns[oc][:].rearrange("o (kc t) -> o kc t", t=9)
                for tap in range(9):
                    if USE_FP8:
                        wt8 = wspool.tile([128, KC * 128], FP8,
                                          tag=f"ws8_{oc}_{tap}",
                                          name=f"ws8_{oc}_{tap}")
                        ws[(oc, tap)] = wt8
                        # both k-chunk transposes land in one PSUM tile so a
                        # single DVE copy drains them (DVE queue pressure)
                        pst = tp_psum.tile([128, KC * 128], BF16, tag="tp",
                                           name="tp")
                        for k in range(KC):
                            nc.tensor.transpose(
                                pst[:, k * 128:(k + 1) * 128],
                                wsign3[:, k * 128:(k + 1) * 128, tap],
                                identity[:])
                        nc.vector.tensor_copy(wt8[:], pst[:])
                    else:
                        for k in range(KC):
                            src = wsign3[:, k * 128:(k + 1) * 128, tap]
                            pst = tp_psum.tile([128, 128], BF16, tag="tp",
                                               name="tp")
                            nc.tensor.transpose(pst[:], src, identity[:])
                            wt = wspool.tile([128, 128], BF16,
                                             tag=f"ws{oc}_{k}_{tap}",
                                             name=f"ws{oc}_{k}_{tap}")
                            nc.vector.tensor_copy(wt[:], pst[:])
                            ws[(oc, k, tap)] = wt

        if nc._stage <= 1:
            # debug cutoff: dump coefs and bail
            nc.sync.dma_start(out=y[0, 0:128, 0, 0:4], in_=ab[:])
            phase_c_pools.close()
            return

        # ---- phase C: binarize into padded planes, then conv ----
        if True:
            def emit_sign(b, k, r0, r1):
                base = k * PLANE_PAD if USE_FP8 else 0
                tgt = xbp[b] if USE_FP8 else xbp[b][k]
                nr = r1 - r0
                lo = base + 1 + (1 + r0) * WP + 1
                interior = (tgt[:, lo:lo + (nr + 1) * WP]
                            .rearrange("p (h w) -> p h w", w=WP)[:, 0:nr, 0:W])
                nc.scalar.activation(
                    interior,
                    xs[b][k][:].rearrange("p (h w) -> p h w", w=W)[:, r0:r1, :],
                    AF.Sign,
                    bias=ab[:, 2 + k:3 + k], scale=ab[:, k:k + 1])

            # row splits, emitted in conv consumption order, so early conv
            # chunks unblock while later planes are still binarizing; the
            # first image is split finest since it gates the conv start
            for r0, r1 in ((0, 32), (32, H)):
                for k in range(KC):
                    emit_sign(0, k, r0, r1)
            for b in range(1, B_LOC):
                for k in range(KC):
                    emit_sign(b, k, 0, 32)
                for k in range(KC):
                    emit_sign(b, k, 32, H)

            if nc._stage <= 2:
                # debug cutoff: read back one row of every xbp plane
                dump = stpool.tile([128, 2 * B_LOC * W], F32, tag="dump",
                                   name="dump")
                for b in range(B_LOC):
                    for k in range(KC):
                        src = (xbp[b][:, k * PLANE_PAD + 60:k * PLANE_PAD + 60 + W]
                               if USE_FP8 else xbp[b][k][:, 60:60 + W])
                        nc.vector.tensor_copy(
                            dump[:, (b * KC + k) * W:(b * KC + k + 1) * W], src)
                nc.sync.dma_start(out=y[0, 0:128, 0:8, :],
                                  in_=dump[:].rearrange("p (r w) -> p r w", w=W))
                phase_c_pools.close()
                return

            for b in range(B_LOC):
                for h0 in range(0, H, R):
                    for oc in range(2):
                        acc = cpsum.tile([128, NF], F32, tag="acc", name="acc")
                        if USE_FP8:
                            xv = xbp[b][:].rearrange("p (i l) -> p i l",
                                                     l=PLANE_PAD)
                            for tap in range(9):
                                dh, dw = tap // 3, tap % 3
                                off = (h0 + dh) * WP + dw
                                lhsT = ws[(oc, tap)][:].rearrange(
                                    "p (i m) -> p i m", m=128)
                                nc.tensor.matmul(
                                    acc[:], lhsT, xv[:, :, off:off + NF],
                                    start=(tap == 0), stop=(tap == 8),
                                    perf_mode=mybir.MatmulPerfMode.DoubleRow)
                        else:
                            i = 0
                            for k in range(KC):
                                for dh in range(3):
                                    for dw in range(3):
                                        off = (h0 + dh) * WP + dw
                                        nc.tensor.matmul(
                                            acc[:],
                                            ws[(oc, k, dh * 3 + dw)][:],
                                            xbp[b][k][:, off:off + NF],
                                            start=(i == 0), stop=(i == 17))
                                        i += 1
                        stage = stpool.tile([128, R, W], F32, tag="stage",
                                            name="stage")
                        accv = (acc[:].rearrange("p (h w) -> p h w", w=WP)
                                [:, :, 1:1 + W])
                        nc.vector.tensor_scalar_mul(stage[:], accv,
                                                    alpha[:, oc:oc + 1])
                        nc.sync.dma_start(
                            out=y[b, oc * 128:(oc + 1) * 128, h0:h0 + R, :],
                            in_=stage[:])
        phase_c_pools.close()


def run_on_hw(x, weight, gamma, beta, **spmd_kwargs):
    nc = build_program()
    in_maps = []
    for i in range(N_CORES):
        in_maps.append({
            "x": np.ascontiguousarray(x[i * B_LOC:(i + 1) * B_LOC]),
            "weight": np.ascontiguousarray(weight),
            "gamma": np.ascontiguousarray(gamma),
            "beta": np.ascontiguousarray(beta),
        })
    return run_bass_kernel_spmd(nc, in_maps, core_ids=list(range(N_CORES)),
                                **spmd_kwargs)


def kernel(x: np.ndarray, weight: np.ndarray, gamma: np.ndarray,
           beta: np.ndarray) -> np.ndarray:
    # The first execution on a freshly-attached device occasionally reports
    # NRT_EXEC_UNIT_UNRECOVERABLE from residue of a prior process; an
    # immediate retry reliably succeeds.
    last_err = None
    for _ in range(3):
        try:
            res = run_on_hw(x, weight, gamma, beta)
            break
        except Exception as e:  # noqa: BLE001 - retry any transient runtime error
            last_err = e
    else:
        raise last_err
    out = np.concatenate([res.results[i]["y"] for i in range(N_CORES)], axis=0)
    return out.astype(np.float32)


if __name__ == "__main__":
    nc = build_program()
    print("build ok:", len(nc.inst_map), "instructions")



# revision 34
# speedup vs baseline: 1.0488x; 1.0488x over previous
"""BinaryLayerWrapper (sync-BN + sign + binarized 3x3 conv) on 8 TRN2 cores.

Strategy (data-parallel, per sharding hint):
  - shard batch B=32 -> 4 images per core; conv weights replicated
  - phase A: stream x shard to SBUF (kept resident), per-channel partial
    sums sum(x), sum(x^2) over local batch+space trailing the DMA stream
  - sync-BN all-reduce of the [128,4] partial stats across the 8 cores:
    a 3-stage XOR-hypercube exchange over direct SBUF->SBUF remote DMAs
    (relative-dest broadcast descriptors, so the one SPMD program needs
    no per-rank addressing).  Single-core builds model the exchange as
    one local SBUF->SBUF DMA hop + reduce, matching the measured cost of
    one remote hop.
  - per-channel a = gamma*rsqrt(var+eps), b = beta - mean*a
  - phase C: xb = Sign(a*x+b) in fp8 written into zero-padded 58x58
    planes; 3x3 conv = 9 fp8 DoubleRow accumulated matmuls per output
    tile (N=464 = 8 output rows x 58 padded cols), then scale by alpha
    and DMA the valid interior out.

The conv math is exact: xb is +-1 (exact in fp8e4m3), weights are
sign(w)/2 = +-0.5 (exact in fp8; the missing 2x is folded into alpha),
products accumulate in fp32 PSUM exactly.

Schedule notes (engine FIFOs execute in emission order, so emission is
chronological per engine):
  - weight DMAs are gated on the end of the x stream via tiny token
    writes, so they cannot displace x bytes (which gate sync-BN) on the
    shared DMA resource; they run during the allreduce+coef window
  - weights transpose directly from f32 (PE), and the PSUM->SBUF drain
    applies sign via one (w>=0)-0.5 tensor_scalar, so no activation-
    engine time is spent on weights: ACT does only stats, sqrt and the
    x sign passes
  - a dummy Sqrt activation at t=0 pins the act-func table that holds
    {sqrt, sign, copy, abs}, avoiding a 1.3us mid-kernel table reload
  - x-sign chunks are split so a conv tile at h0 only depends on sign
    chunks covering image rows <= h0+9 (matmul read spans bleed 2 cols
    into the next row), keeping the conv start fine-grained
  - conv drains alternate DVE/gpsimd; first drains go to DVE interleaved
    with the alpha reduces in ready-order; 6 PSUM banks absorb slack
  - discarded transposes gated on streaming scratch pace the PE through
    the load and bridge phases so the conv starts at full clock
"""

import os

import numpy as np

from concourse import bacc, bass, masks, mybir, tile
from concourse.bass_utils import run_bass_kernel_spmd

F32 = mybir.dt.float32
BF16 = mybir.dt.bfloat16
FP8 = mybir.dt.float8e4

N_CORES = 8
B_LOC = 4          # images per core (32 / 8)
C = 256            # channels (in == out)
KC = 2             # 128-partition channel chunks
H = W = 56
PIX = H * W        # 3136
WP = W + 2         # 58 padded width
PLANE = WP * (H + 2)          # 58*58 = 3364
XBP_LEN = PLANE + 2           # +1 lead pad so all tap offsets are >= 0
PLANE_PAD = 3376              # XBP_LEN rounded to 16 (fp8 DoubleRow Ko step)
R = 8                         # output rows per matmul tile (N=464, 1 PSUM bank)
NF = R * WP                   # 464 matmul free dim
N_TOTAL = 32 * PIX            # full-batch elements per channel (sync-BN)

# sync-BN exchange: XOR-hypercube remote DMAs (1) vs collective_compute (0)
USE_RDMA = os.environ.get("USE_RDMA", "1") == "1"


def build_program(num_devices: int = N_CORES, cc: bool = True,
                  stage: int = 3) -> bass.Bass:
    nc = bacc.Bacc("TRN2", target_bir_lowering=False, debug=False,
                   num_devices=num_devices)
    nc._use_cc = cc
    nc._cc_devices = num_devices
    nc._stage = stage

    x = nc.dram_tensor("x", [B_LOC, C, H, W], F32, kind="ExternalInput").ap()
    w = nc.dram_tensor("weight", [C, C, 3, 3], F32, kind="ExternalInput").ap()
    gamma = nc.dram_tensor("gamma", [C], F32, kind="ExternalInput").ap()
    beta = nc.dram_tensor("beta", [C], F32, kind="ExternalInput").ap()
    y = nc.dram_tensor("y", [B_LOC, C, H, W], F32, kind="ExternalOutput").ap()

    nc._rdma_wait_patches = []
    with tile.TileContext(nc) as tc:
        _body(tc, y, x, w, gamma, beta)
    # The tile scheduler's single-core sim cannot observe remote semaphore
    # increments, so the receive-side folds are emitted without the remote
    # wait and the real semaphore waits are appended here, after scheduling
    # (extra waits can only delay the instruction, never break the schedule).
    for inst, sem, val in nc._rdma_wait_patches:
        si = inst.sync_info or mybir.SyncInfo(on_wait=[], on_update=[])
        nw = mybir.SyncWait(sync_type="semaphore", id=sem.num,
                            ant_name=sem.name, wait_mode="sem-ge-imm",
                            wait_value=val, wait_reg=None)
        inst.sync_info = mybir.SyncInfo(on_wait=list(si.on_wait) + [nw],
                                        on_update=list(si.on_update))
    nc.compile()
    return nc


def _body(tc: tile.TileContext, y, x, w, gamma, beta):
    nc = tc.nc
    add = mybir.AluOpType.add
    mult = mybir.AluOpType.mult
    AF = mybir.ActivationFunctionType
    n_dev = nc._cc_devices
    multi = nc._use_cc and n_dev > 1
    rdma = multi and USE_RDMA
    n_stages = max(1, (n_dev - 1).bit_length()) if rdma else 0

    with (
        tc.tile_pool(name="singles", bufs=1) as singles,
        tc.tile_pool(name="wsbuf", bufs=1) as wspool,
        tc.tile_pool(name="xres", bufs=1) as xpool,
        tc.tile_pool(name="stage", bufs=8) as stpool,
        tc.tile_pool(name="xbp", bufs=1) as xbpool,
        tc.tile_pool(name="dram", bufs=1, space="DRAM") as dram,
        tc.tile_pool(name="cpsum", bufs=5, space="PSUM") as cpsum,
        tc.tile_pool(name="tpps", bufs=2, space="PSUM") as tp_psum,
    ):
        identity = singles.tile([128, 128], BF16, tag="identity")
        masks.make_identity(nc, identity[:])
        identity8 = singles.tile([128, 128], FP8, tag="identity8")
        masks.make_identity(nc, identity8[:])

        # pin the {sqrt, sign, copy, abs} act table before any other
        # activation so it is loaded exactly once, at t=0
        actpin = singles.tile([128, 2], F32, tag="actpin")
        nc.gpsimd.memset(actpin[:, 0:1], 1.0)
        nc.scalar.activation(actpin[:, 1:2], actpin[:, 0:1], AF.Sqrt)

        gb = singles.tile([128, 4], F32, tag="gb")  # gamma k0,k1 | beta k0,k1
        g2 = singles.tile([128, 2], F32, tag="g2")  # gamma^2 per k

        NCH = 10  # stat chunks per k-chunk (3 images x 2 halves + 4 quarters)
        psum_parts = singles.tile([128, KC * NCH], F32, tag="psum_parts")
        psq_parts = singles.tile([128, KC * NCH], F32, tag="psq_parts")
        stats_local = singles.tile([128, 4], F32, tag="stats_local")
        # rx slots for the hypercube exchange + running partials
        rx = singles.tile([128, 12], F32, tag="rx")
        parts = singles.tile([128, 8], F32, tag="parts")  # p1 | p2
        gstats = singles.tile([128, 4], F32, tag="gstats")
        alpha_parts = singles.tile([128, 4], F32, tag="alpha_parts")
        alpha = singles.tile([128, 2], F32, tag="alpha")
        coefs = singles.tile([128, 12], F32, tag="coefs")
        ab = singles.tile([128, 4], F32, tag="ab")  # a k0,k1 | b k0,k1
        junk = singles.tile([128, 4], F32, tag="junk")
        bridge = singles.tile([128, 128], BF16, tag="bridge")

        xs = [[xpool.tile([128, PIX], F32, tag=f"xs{b}_{k}", name=f"xs{b}_{k}")
               for k in range(KC)] for b in range(B_LOC)]
        # per-oc fp8 weights, layout [cin_within_k, (k, tap, cout)]
        ws = [wspool.tile([128, KC * 9 * 128], FP8, tag=f"ws{oc}",
                          name=f"ws{oc}") for oc in range(2)]
        xbp = [xbpool.tile([128, KC * PLANE_PAD], FP8, tag=f"xbp{b}",
                           name=f"xbp{b}") for b in range(B_LOC)]

        if rdma:
            rsems = [nc.alloc_semaphore(name=f"bn_rx{s}")
                     for s in range(n_stages)]
            lsem = nc.alloc_semaphore(name="bn_tx")
            for s in rsems:
                nc.gpsimd.sem_clear(s)
            nc.gpsimd.sem_clear(lsem)

        # zero the halo borders (interior fully overwritten by the sign
        # pass; inter-plane alignment gap never read); DVE+Pool split
        def memset_borders(eng, t, base):
            eng.memset(t[:, base:base + 1], 0.0)
            eng.memset(t[:, base + 1:base + 1 + WP], 0.0)
            eng.memset(t[:, base + 1 + 57 * WP:base + 1 + 57 * WP + WP], 0.0)
            side = (t[:, base + 1 + WP:base + 1 + 57 * WP]
                    .rearrange("p (h w) -> p h w", w=WP))
            eng.memset(side[:, :, 0:1], 0.0)
            eng.memset(side[:, :, WP - 1:WP], 0.0)
            eng.memset(t[:, base + 1 + PLANE:base + 1 + PLANE + 1], 0.0)

        for b in range(B_LOC):
            for k in range(KC):
                eng = nc.vector if (b * KC + k) % 2 == 0 else nc.gpsimd
                memset_borders(eng, xbp[b], k * PLANE_PAD)

        # hypercube exchange descriptors, prepared early (data is read at
        # trigger time); stage s sends the running partial to tpb^(2^s)
        def stage_src(s):
            return stats_local[:] if s == 0 else parts[:, (s - 1) * 4:s * 4]

        if rdma:
            for s in range(n_stages):
                delta = 1 << s
                slot = 4 if (delta & 4) else 0  # cross-die needs slots 4-7
                rdests = [None] * 8
                rdests[slot] = (0, delta)
                nc.gpsimd.remote_dma_broadcast(
                    out_ap=rx[:, s * 4:s * 4 + 4], in_ap=stage_src(s),
                    remote_sem=rsems[s], local_sem=lsem, rdests=rdests)

        with (
            tc.tile_pool(name="wraw", bufs=1) as wraw_pool,
            tc.tile_pool(name="scr", bufs=2) as scr,
            tc.tile_pool(name="scrb", bufs=2) as scrb,
            tc.tile_pool(name="wmps", bufs=1, space="PSUM") as wm_psum,
        ):
            def warm(src):
                # discarded transpose paces PE (p-state keep-warm)
                wt = wm_psum.tile([128, 128], BF16, tag="warm", name="warm")
                nc.tensor.transpose(wt[:], src, identity[:])

            # ---- phase A: x stream + trailing stats; last image in
            # quarter tiles so the post-stream stat tail is short ----
            HPIX = PIX // 2
            QPIX = PIX // 4
            chunks = []
            for b in range(B_LOC - 1):
                for k in range(KC):
                    for hf in range(2):
                        chunks.append((b, k, hf * HPIX, (hf + 1) * HPIX,
                                       k * NCH + b * 2 + hf))
            for k in range(KC):
                for q in range(4):
                    chunks.append((B_LOC - 1, k, q * QPIX, (q + 1) * QPIX,
                                   k * NCH + 6 + q))
            for (b, k, lo, hi, col) in chunks:
                nc.sync.dma_start(
                    out=xs[b][k][:, lo:hi],
                    in_=x[b, k * 128:(k + 1) * 128]
                    .rearrange("c h w -> c (h w)")[:, lo:hi])
                n = hi - lo
                xsl = xs[b][k][:, lo:hi]
                sa = scr.tile([128, HPIX], BF16, tag="scr_a", name="scr_a")
                nc.scalar.activation(sa[:, 0:n], xsl, AF.Copy,
                                     accum_out=psum_parts[:, col:col + 1])
                sb = scrb.tile([128, HPIX], BF16, tag="scr_b", name="scr_b")
                nc.vector.scalar_tensor_tensor(
                    out=sb[:, 0:n], in0=xsl, scalar=1.0, in1=xsl,
                    op0=mult, op1=mult,
                    accum_out=psq_parts[:, col:col + 1])
                warm(sa[:, 0:128])

            # gamma/beta after the x stream so they don't delay it
            nc.sync.dma_start(out=gb[:, 0:2],
                              in_=gamma.rearrange("(k p) -> p k", p=128))
            nc.sync.dma_start(out=gb[:, 2:4],
                              in_=beta.rearrange("(k p) -> p k", p=128))
            nc.gpsimd.tensor_tensor(out=g2[:], in0=gb[:, 0:2], in1=gb[:, 0:2],
                                    op=mult)

            # ---- finalize local stats ----
            nc.vector.tensor_reduce(
                out=stats_local[:, 0:2],
                in_=psum_parts[:].rearrange("p (k n) -> p k n", k=KC),
                axis=mybir.AxisListType.X, op=add)
            nc.vector.tensor_reduce(
                out=stats_local[:, 2:4],
                in_=psq_parts[:].rearrange("p (k n) -> p k n", k=KC),
                axis=mybir.AxisListType.X, op=add)

            # PE pacing through the bridge: tiny copies of allreduce/coef
            # products into the bridge tile give freshly-written warm gates
            def bridge_warm(i, gate):
                nc.vector.tensor_copy(bridge[:, 4 * i:4 * i + 4], gate)
                warm(bridge[:, 0:128])

            # ---- sync-BN exchange ----
            if rdma:
                # per stage: a Pool read of the stage source orders the
                # trigger after the data write; DVE waits the remote sem
                # (+2 per arrived send) then folds the received slot in
                for s in range(n_stages):
                    acc_in = stage_src(s)
                    acc_out = (parts[:, s * 4:s * 4 + 4] if s < n_stages - 1
                               else gstats[:])
                    nc.gpsimd.tensor_copy(junk[:, s:s + 1], acc_in[:, 0:1])
                    nc.gpsimd.trigger_dma(1)
                    bi = nc.vector.tensor_tensor(
                        out=acc_out, in0=acc_in,
                        in1=rx[:, s * 4:s * 4 + 4], op=add)
                    nc._rdma_wait_patches.append(
                        (bi.ins, rsems[s], 2 * (s + 1)))
                    if s == 0:
                        bridge_warm(0, acc_out[:, 0:4])
            elif multi:
                ccin = dram.tile([128, 4], F32, tag="ccin", name="ccin")
                ccout = dram.tile([128, 4], F32, tag="ccout", name="ccout")
                nc.sync.dma_start(out=ccin[:], in_=stats_local[:])
                nc.gpsimd.collective_compute(
                    "AllReduce", add,
                    replica_groups=[list(range(n_dev))],
                    ins=[ccin.opt()], outs=[ccout.opt()])
                nc.sync.dma_start(out=gstats[:], in_=ccout[:])
            else:
                # single-core stand-in for the hypercube exchange: one
                # local SBUF->SBUF hop + fold, mirroring one remote stage
                nc.sync.dma_start(out=rx[:, 0:4], in_=stats_local[:])
                nc.vector.tensor_scalar_add(gstats[:], rx[:, 0:4], 0.0)
                bridge_warm(0, rx[:, 0:4])

            # ---- weight DMA: token writes gated on the local stats keep
            # the 8 sub-chunks strictly after the x stream on the shared
            # DMA resource (the resource is granted in request order) ----
            wraws = []
            for oc in range(2):
                wraw = wraw_pool.tile([128, C * 9], F32, tag=f"wraw{oc}",
                                      name=f"wraw{oc}")
                wraws.append(wraw)
            for oc in range(2):
                for i in range(4):
                    nc.gpsimd.tensor_copy(wraws[oc][:, i * 576:i * 576 + 1],
                                          stats_local[:, 0:1])
            for oc in range(2):
                wsrc = w[oc * 128:(oc + 1) * 128].rearrange(
                    "o c kh kw -> o (c kh kw)")
                for i in range(4):
                    sl = slice(i * 576, (i + 1) * 576)
                    nc.sync.dma_start(out=wraws[oc][:, sl], in_=wsrc[:, sl])

            bridge_warm(1, gstats[:, 0:4])

            # ---- BN coefficients: a = sqrt(gamma^2 / (var+eps)),
            # b = beta - mean*a ----
            mean = coefs[:, 0:2]
            msq = coefs[:, 2:4]
            m2 = coefs[:, 4:6]
            var = coefs[:, 6:8]
            rec = coefs[:, 10:12]
            nc.vector.tensor_scalar_mul(coefs[:, 0:4], gstats[:], 1.0 / N_TOTAL)
            nc.vector.tensor_tensor(out=m2, in0=mean, in1=mean, op=mult)
            nc.vector.scalar_tensor_tensor(
                out=var, in0=msq, scalar=1e-5, in1=m2,
                op0=add, op1=mybir.AluOpType.subtract)
            nc.vector.reciprocal(rec, var)
            for k in range(KC):
                nc.scalar.activation(ab[:, k:k + 1], rec[:, k:k + 1], AF.Sqrt,
                                     scale=g2[:, k:k + 1])
            nc.vector.tensor_tensor(out=coefs[:, 4:6], in0=mean,
                                    in1=ab[:, 0:2], op=mult)
            nc.vector.tensor_tensor(out=ab[:, 2:4], in0=gb[:, 2:4],
                                    in1=coefs[:, 4:6],
                                    op=mybir.AluOpType.subtract)
            bridge_warm(2, ab[:, 0:4])

            # ---- weight prep, per (oc,k) chunk as its DMA lands: Pool
            # turns w into sign(w)/2 = +-0.5 fp8 in SBUF (alpha carries
            # the 2x; gpsimd cannot touch PSUM), PE transposes the fp8
            # into one [128,1152] PSUM tile per (oc,k).  Only DVE/ACT can
            # read PSUM back: DVE drains oc0 (it gates the conv start),
            # ACT drains oc1 after the image-0 signs (needed ~6us later)
            w05s = []
            tgroups = [(0, 4), (4, 8), (8, 9)]
            for oc in range(2):
                w05 = wraw_pool.tile([128, C * 9], BF16, tag=f"w05_{oc}",
                                     name=f"w05_{oc}")
                w05s.append(w05)
            for oc in range(2):
                for k in range(KC):
                    sl = slice(k * 1152, (k + 1) * 1152)
                    nc.gpsimd.tensor_scalar(
                        out=w05s[oc][:, sl], in0=wraws[oc][:, sl],
                        scalar1=0.0, scalar2=0.5,
                        op0=mybir.AluOpType.is_ge,
                        op1=mybir.AluOpType.subtract)

            def wprep(oc, k, drain):
                # transpose one (oc,k) chunk into PSUM tap-groups and
                # drain them to the fp8 lhsT tile on the given engine
                w3 = w05s[oc][:].rearrange("o (c t) -> o c t", t=9)
                for (t0, t1) in tgroups:
                    pool = tp_psum if t1 - t0 == 4 else wm_psum
                    pst = pool.tile([128, (t1 - t0) * 128], BF16,
                                    tag="warm" if t1 - t0 == 1 else "tp4",
                                    name="tp")
                    for t in range(t0, t1):
                        nc.tensor.transpose(
                            pst[:, (t - t0) * 128:(t - t0 + 1) * 128],
                            w3[:, k * 128:(k + 1) * 128, t],
                            identity[:])
                    dst = ws[oc][:, (k * 9 + t0) * 128:(k * 9 + t1) * 128]
                    if drain == "dve":
                        nc.vector.tensor_copy(dst, pst[:])
                    else:
                        nc.scalar.activation(dst, pst[:], AF.Copy)

            # oc0 gates the conv start: prep it now, drains on DVE
            wprep(0, 0, "dve")
            wprep(0, 1, "dve")

            # ---- phase C: binarize into padded planes + conv ----
            def emit_sign(b, k, r0, r1):
                base = k * PLANE_PAD
                nr = r1 - r0
                lo = base + 1 + (1 + r0) * WP + 1
                interior = (xbp[b][:, lo:lo + (nr + 1) * WP]
                            .rearrange("p (h w) -> p h w", w=WP)[:, 0:nr, 0:W])
                nc.scalar.activation(
                    interior,
                    xs[b][k][:].rearrange("p (h w) -> p h w", w=W)[:, r0:r1, :],
                    AF.Sign,
                    bias=ab[:, 2 + k:3 + k], scale=ab[:, k:k + 1])

            splits = {0: ((0, 11), (11, 27), (27, 43), (43, H))}
            for b in range(1, B_LOC):
                splits[b] = ((0, 29), (29, H))
            # image 0 signs up front (they gate the conv start); later
            # images' sign pairs are interleaved into the conv emission so
            # the ACT queue stays chronological with its drains
            for rr in splits[0]:
                for k in range(KC):
                    emit_sign(0, k, rr[0], rr[1])
            # oc1 weight prep and later images' sign pairs interleave into
            # the conv emission so the PE/ACT FIFOs stay chronological
            sign_at = {}
            for b in range(1, B_LOC):
                base_ti = 14 * (b - 1)
                sign_at[base_ti + 6] = (b, splits[b][0])
                sign_at[base_ti + 9] = (b, splits[b][1])

            # alpha = 2 * mean|w| per oc (2x compensates the +-0.5
            # weights); pieces gated per (oc,k) DMA chunk
            def alpha_piece(oc, k):
                nc.vector.tensor_reduce(
                    out=alpha_parts[:, oc * 2 + k:oc * 2 + k + 1],
                    in_=wraws[oc][:, k * 1152:(k + 1) * 1152],
                    axis=mybir.AxisListType.X, op=add,
                    apply_absolute_value=True)

            def alpha_comb(oc):
                nc.vector.tensor_reduce(
                    out=coefs[:, 8 + oc:9 + oc],
                    in_=alpha_parts[:, oc * 2:oc * 2 + 2],
                    axis=mybir.AxisListType.X, op=add)
                nc.vector.tensor_scalar_mul(alpha[:, oc:oc + 1],
                                            coefs[:, 8 + oc:9 + oc],
                                            2.0 / (C * 9))

            alpha_piece(0, 0)
            alpha_piece(0, 1)
            alpha_comb(0)

            # conv tiles; image 0 runs all oc=0 first (oc=1 lhsT tiles
            # land later), later images interleave
            tiles = []
            for oc in range(2):
                for h0 in range(0, H, R):
                    tiles.append((0, h0, oc))
            for b in range(1, B_LOC):
                for h0 in range(0, H, R):
                    for oc in range(2):
                        tiles.append((b, h0, oc))

            if nc._stage <= 2:
                nc.sync.dma_start(out=y[0, 0:128, 0, 0:4], in_=ab[:])
                return

            for ti, (b, h0, oc) in enumerate(tiles):
                if ti in sign_at:
                    sb_, rr = sign_at[ti]
                    for k in range(KC):
                        emit_sign(sb_, k, rr[0], rr[1])
                acc = cpsum.tile([128, NF], F32, tag="acc", name="acc")
                xv = xbp[b][:].rearrange("p (i l) -> p i l", l=PLANE_PAD)
                lhsT = ws[oc][:].rearrange("p (i t m) -> p i t m", i=KC, m=128)
                for tap in range(9):
                    dh, dw = tap // 3, tap % 3
                    off = (h0 + dh) * WP + dw
                    nc.tensor.matmul(
                        acc[:], lhsT[:, :, tap, :], xv[:, :, off:off + NF],
                        start=(tap == 0), stop=(tap == 8),
                        perf_mode=mybir.MatmulPerfMode.DoubleRow)
                stage = stpool.tile([128, R, W], F32, tag="stage", name="stage")
                accv = (acc[:].rearrange("p (h w) -> p h w", w=WP)[:, :, 1:1 + W])
                if ti == 2:
                    alpha_piece(1, 0)
                    wprep(1, 0, "act")
                if ti == 4:
                    alpha_piece(1, 1)
                    alpha_comb(1)
                    wprep(1, 1, "act")
                if ti < 6 or ti % 2 == 0:
                    nc.vector.tensor_scalar_mul(stage[:], accv,
                                                alpha[:, oc:oc + 1])
                else:
                    nc.scalar.activation(stage[:], accv, AF.Copy,
                                         scale=alpha[:, oc:oc + 1])
                nc.sync.dma_start(
                    out=y[b, oc * 128:(oc + 1) * 128, h0:h0 + R, :],
                    in_=stage[:])


def run_on_hw(x, weight, gamma, beta, **spmd_kwargs):
    nc = build_program()
    in_maps = []
    for i in range(N_CORES):
        in_maps.append({
            "x": np.ascontiguousarray(x[i * B_LOC:(i + 1) * B_LOC]),
            "weight": np.ascontiguousarray(weight),
            "gamma": np.ascontiguousarray(gamma),
            "beta": np.ascontiguousarray(beta),
        })
    return run_bass_kernel_spmd(nc, in_maps, core_ids=list(range(N_CORES)),
                                **spmd_kwargs)


def kernel(x: np.ndarray, weight: np.ndarray, gamma: np.ndarray,
           beta: np.ndarray) -> np.ndarray:
    # The first execution on a freshly-attached device occasionally reports
    # NRT_EXEC_UNIT_UNRECOVERABLE from residue of a prior process; an
    # immediate retry reliably succeeds.
    last_err = None
    for _ in range(3):
        try:
            res = run_on_hw(x, weight, gamma, beta)
            break
        except Exception as e:  # noqa: BLE001 - retry any transient runtime error
            last_err = e
    else:
        raise last_err
    out = np.concatenate([res.results[i]["y"] for i in range(N_CORES)], axis=0)
    return out.astype(np.float32)


if __name__ == "__main__":
    nc = build_program()
    print("build ok:", len(nc.inst_map), "instructions")
